# revision 1
# baseline (speedup 1.0000x reference)
"""Trainium2 Bass kernel for nn_ASSA (sparse+dense dual-branch attention).

Reference computation (B=4, N=2048, C=512, H=8, D=64):
    qkv = x @ w_qkv -> q,k,v (B,H,N,D)
    attn = (q @ k^T) * D^-0.5
    sparse  = relu(attn)^2 / (rowsum + 1e-8)
    dense   = softmax(attn)
    final   = (a/wsum)*sparse + (b/wsum)*dense      a=sig(alpha), b=sig(beta)
    out = ((final @ v).transpose.reshape(B,N,C)) @ w_proj + b_proj

Sharding: 8 cores = batch(4) x head-group(2). Each core: 1 batch, 4 heads
(2 head-pairs). All compute in transposed score layout S^T[kt, qt] so both
branch row-reductions become PE ones-matmuls, PV matmuls are full-K, and
per-(head,branch) row normalization is applied pre-projection via a PE
selector-broadcast matmul.
"""

import os
import sys
import math
import numpy as np

for _p in ("/opt/trn_rl_repo",):
    if _p not in sys.path:
        sys.path.insert(0, _p)

from contextlib import ExitStack

import concourse.bass as bass
import concourse.bacc as bacc_mod
import concourse.tile as tile
from concourse import mybir
from concourse._compat import with_exitstack
from concourse.tile_rust import add_dep_helper

F32 = mybir.dt.float32
F32R = mybir.dt.float32r
BF16 = mybir.dt.bfloat16
AF = mybir.ActivationFunctionType
ALU = mybir.AluOpType

B, N, C = 4, 2048, 512
H, D = 8, 64
NCORES = 8
HEADS_PER_CORE = 4  # 2 head-pairs
EPS = 1e-8

# qt chunk per head processed per attention block
QC = 512
N_QC = N // QC          # 4
N_KT = N // 128         # 16

# Fraction of square ops on DVE vs GPSIMD: kt % 4 == 0 -> DVE, else GPSIMD
SQUARE_DVE_MOD = int(os.environ.get("KBENCH_SQ_DVE_MOD", "3"))
USE_GPSIMD_SQ = os.environ.get("KBENCH_GPSIMD_SQ", "1") == "1"
REPEAT = int(os.environ.get("KBENCH_REPEAT", "1"))
RELU_ACT_MOD = int(os.environ.get("KBENCH_RELU_ACT_MOD", "3"))
PV65 = os.environ.get("KBENCH_PV65", "0") == "1"
NO_ELEM = os.environ.get("KBENCH_NO_ELEM", "0") == "1"
MIX = os.environ.get("KBENCH_MIX", "C")
ABL = os.environ.get("KBENCH_ABL", "")


def f32r(ap):
    return ap.bitcast(F32R)


@with_exitstack
def assa_kernel(ctx: ExitStack, tc: tile.TileContext):
    nc = tc.nc
    _prev_pe = [None]

    def pe_mm(*args, **kw):
        inst = nc.tensor.matmul(*args, **kw)
        if _prev_pe[0] is not None:
            add_dep_helper(inst.ins, _prev_pe[0].ins, sync=False, reason="pe order")
        _prev_pe[0] = inst
        return inst

    # ---------- DRAM I/O ----------
    xT_d = nc.declare_dram_parameter("xT", [C, N], F32R, isOutput=False)
    wqk_d = nc.declare_dram_parameter("w_qk", [C, 512], F32R, isOutput=False)
    wv_d = nc.declare_dram_parameter("w_v", [C, 256], F32R, isOutput=False)
    wproj_d = nc.declare_dram_parameter("w_proj", [256, 512], F32R, isOutput=False)
    # consts: cols 0:128 sel_sp, 128:256 sel_ex, 256 eps_col, 257 cinv_col
    consts_d = nc.declare_dram_parameter("consts", [128, 260], F32R, isOutput=False)
    out_d = nc.declare_dram_parameter("out", [N, 512], F32, isOutput=True)

    # ---------- SBUF pools (persistent) ----------
    pw = ctx.enter_context(tc.tile_pool(name="weights", bufs=1))
    px = ctx.enter_context(tc.tile_pool(name="xT", bufs=1))
    pqk = ctx.enter_context(tc.tile_pool(name="qkT", bufs=1))
    pv_sb = ctx.enter_context(tc.tile_pool(name="v_sb", bufs=1))
    pout = ctx.enter_context(tc.tile_pool(name="outT", bufs=1))
    prs = ctx.enter_context(tc.tile_pool(name="rs", bufs=1))
    pelem = ctx.enter_context(tc.tile_pool(name="elem", bufs=3))
    pmisc = ctx.enter_context(tc.tile_pool(name="misc", bufs=2))

    # weights
    w_qk = [pw.tile([128, 512], F32R, name=f"wqk{c}", tag=f"wqk{c}") for c in range(4)]
    for c in range(4):
        nc.sync.dma_start(w_qk[c][:], wqk_d[c * 128:(c + 1) * 128, :])
    w_v = [pw.tile([128, 256], F32R, name=f"wv{c}", tag=f"wv{c}") for c in range(4)]
    for c in range(4):
        nc.sync.dma_start(w_v[c][:], wv_d[c * 128:(c + 1) * 128, :])
    w_proj = [pw.tile([128, 512], F32R, name=f"wp{c}", tag=f"wp{c}") for c in range(2)]
    for c in range(2):
        nc.sync.dma_start(w_proj[c][:], wproj_d[c * 128:(c + 1) * 128, :])
    consts = pw.tile([128, 260], F32R, tag="consts")
    nc.sync.dma_start(consts[:], consts_d[:, :])
    ones_bf = pw.tile([128, 32], BF16, tag="ones")
    nc.vector.memset(ones_bf[:], 1.0)

    # warm the Exp table early (scratch)
    warm = pw.tile([128, 8], F32, tag="warm")
    nc.scalar.activation(warm[:], consts[:, 0:8].bitcast(F32), AF.Exp)

    # x^T: 4 chunk tiles [128, 2048]
    xT = [px.tile([128, N], F32R, name=f"xT{c}", tag=f"xT{c}") for c in range(4)]
    for c in range(4):
        nc.sync.dma_start(xT[c][:], xT_d[c * 128:(c + 1) * 128, :])

    # qkT: tiles [128, 2048]: 0=q hp0, 1=q hp1, 2=k hp0, 3=k hp1
    qkT = [pqk.tile([128, N], F32, name=f"qkT{m}", tag=f"qkT{m}") for m in range(4)]
    # v natural layout, bf16. PV65: per token-chunk 4x(64 v cols + ones col)
    VCH = 260 if PV65 else 256
    v_sb = pv_sb.tile([128, 16 * VCH], BF16, tag="v")
    if PV65:
        nc.vector.memset(v_sb[:], 1.0)

    # ---------- optional HW repeat loop (benchmark) ----------
    import contextlib
    rep_cm = tc.For_i(0, REPEAT, 1) if REPEAT > 1 else contextlib.nullcontext()
    ctx.enter_context(rep_cm)

    # ---------- Phase 1: qkT = (x @ w_qk)^T ----------
    with tc.tile_pool(name="ps1", bufs=2, space="PSUM") as ps1:
        # 1x1 pre-wait matmuls: one per DMA-loaded tile so no real (fused-
        # weight-load) matmul ever needs more than one sync wait.
        dummy_ps = ps1.tile([32, 32], F32, tag="dummy", bufs=1)
        for i, tl in enumerate(w_qk + w_v + w_proj + [consts] + xT):
            pe_mm(dummy_ps[0:32, 0:32], tl[0:32, 0:32], tl[0:32, 0:32],
                  start=True, stop=True, skip_group_check=True)
        for m in range(4):
            for t in range(4):  # token chunks of 512
                acc = ps1.tile([128, 512], F32, tag="qk_acc")
                for c in range(4):
                    pe_mm(
                        acc[:],
                        w_qk[c][:, m * 128:(m + 1) * 128],
                        xT[c][:, t * 512:(t + 1) * 512],
                        start=(c == 0), stop=(c == 3),
                    )
                nc.scalar.copy(f32r(qkT[m][:, t * 512:(t + 1) * 512]), acc[:])

        # ---------- Phase 2: v = x @ w_v (natural layout) ----------
        for t in range(16):  # token chunks of 128
            acc = ps1.tile([128, 256], F32, tag="v_acc")
            for c in range(4):
                pe_mm(
                    acc[:],
                    xT[c][:, t * 128:(t + 1) * 128],
                    w_v[c][:],
                    start=(c == 0), stop=(c == 3),
                )
            if PV65:
                dst = v_sb[:, t * VCH:t * VCH + VCH].rearrange(
                    "p (h c) -> p h c", h=4)[:, :, 0:64]
                nc.vector.tensor_copy(dst, acc[:].rearrange("p (h c) -> p h c", h=4))
            else:
                nc.vector.tensor_copy(v_sb[:, t * 256:(t + 1) * 256], acc[:])

    # outputs of attention: per hp, per branch [128, 2048] fp32
    outT_sp = [pout.tile([128, N], F32, name=f"osp{hp}", tag=f"osp{hp}") for hp in range(2)]
    outT_ex = [pout.tile([128, N], F32, name=f"oex{hp}", tag=f"oex{hp}") for hp in range(2)]
    rs_stage = [prs.tile([128, N], F32, name=f"rss{hp}", tag=f"rss{hp}") for hp in range(2)]
    if PV65:
        # only 4 stream-rows get written; init the rest so the downstream
        # eps/recip pass reads defined values (selector zeroes them anyway)
        for hp in range(2):
            nc.vector.memset(rs_stage[hp][:], 1.0)

    # ---------- Phase 3: attention (software-pipelined) ----------
    # PE stream order per step: scores(k+1) BEFORE PV/rs(k), so the in-order
    # PE queue never head-blocks on the elementwise chain of step k.
    with (
        tc.tile_pool(name="ps_s", bufs=2, space="PSUM") as ps_s,
        tc.tile_pool(name="ps_pv", bufs=1, space="PSUM") as ps_pv,
    ):
        steps = [(hp, qc, kt)
                 for hp in range(2) for qc in range(N_QC) for kt in range(N_KT)]
        chunk_tiles = {}
        pend_ex = None
        pend_sp = []
        _stale = []
        _pair = [None]

        def emit_ex(p):
            hp, qc, kt, ex_t = p
            first = kt == 0
            last = kt == N_KT - 1
            _, pv_ex, rs_ps = chunk_tiles[(hp, qc)]
            for h2, c0 in ((0, 0), (1, 64)):
                vt = v_sb[:, kt * 128 * 2 + (hp * 2 + h2) * 64:
                          kt * 128 * 2 + (hp * 2 + h2) * 64 + 64]
                pe_mm(
                    pv_ex[c0:c0 + 64, :], vt,
                    ex_t[:, h2 * 512:h2 * 512 + 512],
                    start=first, stop=last,
                    tile_position=(0, c0), skip_group_check=True,
                )
            for i, srct in ((2, ex_t[:, 0:512]), (3, ex_t[:, 512:1024])):
                pe_mm(
                    rs_ps[32 * i:32 * i + 32, :],
                    ones_bf[:], srct,
                    start=first, stop=last,
                    tile_position=(0, 32 * i), skip_group_check=True,
                )

        def emit_sp(p):
            hp, qc, kt, sp_t = p
            q0 = qc * QC
            first = kt == 0
            last = kt == N_KT - 1
            pv_sp, pv_ex, rs_ps = chunk_tiles[(hp, qc)]
            for h2, c0 in ((0, 0), (1, 64)):
                vt = v_sb[:, kt * 128 * 2 + (hp * 2 + h2) * 64:
                          kt * 128 * 2 + (hp * 2 + h2) * 64 + 64]
                pe_mm(
                    pv_sp[c0:c0 + 64, :], vt,
                    sp_t[:, h2 * 512:h2 * 512 + 512],
                    start=first, stop=last,
                    tile_position=(0, c0), skip_group_check=True,
                )
            for i, srct in ((0, sp_t[:, 0:512]), (1, sp_t[:, 512:1024])):
                pe_mm(
                    rs_ps[32 * i:32 * i + 32, :],
                    ones_bf[:], srct,
                    start=first, stop=last,
                    tile_position=(0, 32 * i), skip_group_check=True,
                )
            if last:
                nc.scalar.copy(outT_sp[hp][:, q0:q0 + QC], pv_sp[:])
                nc.scalar.copy(outT_ex[hp][:, q0:q0 + QC], pv_ex[:])
                nc.vector.tensor_copy(rs_stage[hp][:, q0:q0 + QC], rs_ps[:])
                del chunk_tiles[(hp, qc)]

        for si, (hp, qc, kt) in enumerate(steps):
            qT = qkT[hp]      # [128 = 2 heads x 64, 2048]
            kT = qkT[2 + hp]
            q0 = qc * QC
            k0 = kt * 128
            if kt == 0 and pend_sp:
                # flush previous chunk so its PSUM tiles free before reuse
                if pend_ex is not None:
                    emit_ex(pend_ex)
                    pend_ex = None
                for p in pend_sp:
                    emit_sp(p)
                pend_sp = []
            if kt == 0:
                if PV65:
                    chunk_tiles[(hp, qc)] = tuple(
                        ps_pv.tile([128, QC], F32, name=f"pv{j}", tag=f"pv{j}")
                        for j in range(4))
                else:
                    chunk_tiles[(hp, qc)] = (
                        ps_pv.tile([128, QC], F32, name="pv_sp", tag="pv_sp"),
                        ps_pv.tile([128, QC], F32, name="pv_ex", tag="pv_ex"),
                        ps_pv.tile([128, QC], F32, name="rs_ps", tag="rs"),
                    )
            # scores fp32, 2 row-packed MMs into 2 adjacent banks
            s_ps = ps_s.tile([128, 1024], F32, tag="s")
            pe_mm(
                s_ps[:, 0:512],
                f32r(kT[0:64, k0:k0 + 128]),
                f32r(qT[0:64, q0:q0 + QC]),
                start=True, stop=True,
                tile_position=(0, 0), skip_group_check=True,
            )
            pe_mm(
                s_ps[:, 512:1024],
                f32r(kT[64:128, k0:k0 + 128]),
                f32r(qT[64:128, q0:q0 + QC]),
                start=True, stop=True,
                tile_position=(64, 0), skip_group_check=True,
            )

            if NO_ELEM and si > 0:
                ex_t, sp_t = _stale[0]
            elif ABL == "noexp":
                sp_t = pelem.tile([128, 1024], BF16, tag="sp", bufs=4)
                relu_t = pelem.tile([128, 1024], BF16, tag="relu")
                mode = MIX[si % len(MIX)]
                if mode in ("A", "D"):
                    nc.vector.tensor_scalar_max(relu_t[:], s_ps[:], 0.0)
                else:
                    nc.scalar.activation(relu_t[:], s_ps[:], AF.Relu)
                if mode in ("A", "B"):
                    nc.gpsimd.tensor_mul(sp_t[:], relu_t[:], relu_t[:])
                else:
                    nc.vector.tensor_mul(sp_t[:], relu_t[:], relu_t[:])
                ex_t = sp_t
            elif ABL == "norelu":
                ex_t = pelem.tile([128, 1024], BF16, tag="ex")
                nc.scalar.activation(ex_t[:], s_ps[:], AF.Exp)
                sp_t = ex_t
            else:
                ex_t = pelem.tile([128, 1024], BF16, tag="ex")
                nc.scalar.activation(ex_t[:], s_ps[:], AF.Exp)

                sp_t = pelem.tile([128, 1024], BF16, tag="sp", bufs=4)
                relu_t = pelem.tile([128, 1024], BF16, tag="relu")
                mode = MIX[si % len(MIX)]
                if mode == "A":   # DVE relu + gpsimd square
                    nc.vector.tensor_scalar_max(relu_t[:], s_ps[:], 0.0)
                    nc.gpsimd.tensor_mul(sp_t[:], relu_t[:], relu_t[:])
                elif mode == "C":  # ACT relu + DVE square
                    nc.scalar.activation(relu_t[:], s_ps[:], AF.Relu)
                    nc.vector.tensor_mul(sp_t[:], relu_t[:], relu_t[:])
                elif mode == "B":  # ACT relu + gpsimd square
                    nc.scalar.activation(relu_t[:], s_ps[:], AF.Relu)
                    nc.gpsimd.tensor_mul(sp_t[:], relu_t[:], relu_t[:])
                else:              # "D": DVE relu + DVE square
                    nc.vector.tensor_scalar_max(relu_t[:], s_ps[:], 0.0)
                    nc.vector.tensor_mul(sp_t[:], relu_t[:], relu_t[:])
                if NO_ELEM:
                    _stale.append((ex_t, sp_t))

            if pend_ex is not None:
                emit_ex(pend_ex)
            if len(pend_sp) >= 2:
                emit_sp(pend_sp.pop(0))
            pend_ex = (hp, qc, kt, ex_t)
            pend_sp.append((hp, qc, kt, sp_t))
        emit_ex(pend_ex)
        for p in pend_sp:
            emit_sp(p)

    # ---------- Phase 4: normalize, combine, project ----------
    # g = 1 / ((rs + eps_col) * cinv_col); row r holds stream r//32
    g_all = []
    for hp in range(2):
        den = pmisc.tile([128, N], F32, name=f"den{hp}", tag=f"den{hp}", bufs=1)
        nc.vector.tensor_scalar(
            den[:], rs_stage[hp][:],
            consts[:, 256:257].bitcast(F32), consts[:, 257:258].bitcast(F32),
            op0=ALU.add, op1=ALU.mult,
        )
        g = pmisc.tile([128, N], F32R, name=f"g{hp}", tag=f"g{hp}", bufs=1)
        with nc.allow_low_precision(reason="f32r rounding of normalization factors"):
            nc.vector.reciprocal(g[:], den[:])
        g_all.append(g)

    with tc.tile_pool(name="ps4", bufs=2, space="PSUM") as ps4:
        # combine: comb[hp] = outT_sp * gb_sp + outT_ex * gb_ex  (into outT_sp)
        for hp in range(2):
            for qc in range(N_QC):
                q0 = qc * QC
                gb_sp = ps4.tile([128, QC], F32, tag="gb_sp")
                gb_ex = ps4.tile([128, QC], F32, tag="gb_ex")
                pe_mm(
                    gb_sp[:], consts[:, 0:128],
                    g_all[hp][:, q0:q0 + QC], start=True, stop=True,
                )
                pe_mm(
                    gb_ex[:], consts[:, 128:256],
                    g_all[hp][:, q0:q0 + QC], start=True, stop=True,
                )
                t1 = pmisc.tile([128, QC], F32, tag="t1")
                nc.vector.tensor_mul(t1[:], outT_sp[hp][:, q0:q0 + QC], gb_sp[:])
                t2 = pmisc.tile([128, QC], F32, tag="t2")
                nc.vector.tensor_mul(t2[:], outT_ex[hp][:, q0:q0 + QC], gb_ex[:])
                nc.vector.tensor_add(f32r(qkT[hp][:, q0:q0 + QC]), t1[:], t2[:])

        # projection: out[t*128:(t+1)*128, :] = sum_hp comb[hp][:, t].T @ w_proj[hp]
        for t in range(16):
            acc = ps4.tile([128, 512], F32, tag="proj")
            for hp in range(2):
                pe_mm(
                    acc[:],
                    f32r(qkT[hp][:, t * 128:(t + 1) * 128]),
                    w_proj[hp][:],
                    start=(hp == 0), stop=(hp == 1),
                )
            fin = pmisc.tile([128, 512], F32, tag="fin")
            nc.scalar.copy(fin[:], acc[:])
            nc.sync.dma_start(out_d[t * 128:(t + 1) * 128, :], fin[:])


_CACHE = {}


def _build():
    if "nc" in _CACHE:
        return _CACHE["nc"]
    nc = bacc_mod.Bacc()
    with tile.TileContext(nc) as tc:
        assa_kernel(tc)
    nc.finalize()
    _CACHE["nc"] = nc
    return nc


def _host_prep(x, w_qkv, alpha, beta, w_proj):
    """Build per-core input maps."""
    scale = D ** -0.5
    a = 1.0 / (1.0 + math.exp(-float(alpha[0])))
    b = 1.0 / (1.0 + math.exp(-float(beta[0])))
    wsum = a + b + EPS
    c1, c2 = a / wsum, b / wsum

    w3 = np.asarray(w_qkv).reshape(C, 3, H, D)
    x = np.asarray(x)
    w_proj = np.asarray(w_proj)

    # constants tile
    sel_sp = np.zeros((128, 128), np.float32)
    sel_ex = np.zeros((128, 128), np.float32)
    for m in range(128):
        sel_sp[32 * (m // 64), m] = 1.0
        sel_ex[64 + 32 * (m // 64), m] = 1.0
    eps_col = np.zeros((128, 1), np.float32)
    eps_col[0:64] = EPS
    cinv_col = np.zeros((128, 1), np.float32)
    cinv_col[0:64] = 1.0 / c1
    cinv_col[64:128] = 1.0 / c2
    consts = np.concatenate(
        [sel_sp, sel_ex, eps_col, cinv_col, np.zeros((128, 2), np.float32)], axis=1
    )

    in_maps = []
    for core in range(NCORES):
        bb, g = core // 2, core % 2
        hs = slice(4 * g, 4 * g + 4)
        wq = w3[:, 0, hs, :].reshape(C, 256)
        wk = (w3[:, 1, hs, :] * scale).reshape(C, 256)
        in_maps.append({
            "xT": np.ascontiguousarray(x[bb].T),
            "w_qk": np.ascontiguousarray(np.concatenate([wq, wk], axis=1)),
            "w_v": np.ascontiguousarray(w3[:, 2, hs, :].reshape(C, 256)),
            "w_proj": np.ascontiguousarray(w_proj[g * 256:(g + 1) * 256, :]),
            "consts": consts,
        })
    return in_maps


def kernel(x, w_qkv, alpha, beta, w_proj, b_proj):
    from concourse.bass_utils import run_bass_kernel_spmd

    nc = _build()
    in_maps = _host_prep(x, w_qkv, alpha, beta, w_proj)
    core_ids = list(range(NCORES))
    trace = os.environ.get("KBENCH_TRACE", "0") == "1"
    res = run_bass_kernel_spmd(nc, in_maps, core_ids, trace=trace)
    if trace:
        _CACHE["last_results"] = res

    out = np.zeros((B, N, C), np.float32)
    for bb in range(B):
        out[bb] = res.results[2 * bb]["out"] + res.results[2 * bb + 1]["out"]
    out += np.asarray(b_proj)[None, None, :]
    return out

